# revision 49
# baseline (speedup 1.0000x reference)
"""Multi-head attention (B=4, S=2048, D=1024, H=16, Dh=64) on 8 NeuronCores.

Sharding: core c handles batch b=c//2 and head-group g=c%2 (8 heads).
wq/wk/wv column-parallel, wo row-parallel; host sums the two partial
wo-products per batch and adds bo.

v2 design (per-core):
  - Projections (bf16 matmuls, f32 PSUM): V with ones-row bias matmul;
    Q/K packed q|k into one [128,1024] PSUM tile, bias+copy on Pool.
  - Causal flash attention per (head-pair, 512-wide q strip): scores for
    both heads land in one [128,1024] PSUM tile (A cols 0-511, B cols
    512-1023), ONE exp per k-block on ACT covering [c0:1024]; causal
    diag masked post-exp with a single [128,640] bf16 DVE multiply
    (tri|ones|tri frame). PV accumulates [v|1]^T @ at into [65,512]
    PSUM per head (row 64 = softmax denominator).
  - Normalization off the critical path: raw [65,512] PSUM -> bf16
    staging (frees PSUM), PE broadcast of the den row, DVE
    reciprocal_approx_fast, fused mul+cast into aout. Head B's half is
    moved to partitions 64-127 via a Pool DIRECT2D copy.
  - Wo deferred: accumulate all 4 head-pairs (K=512) in PSUM, single
    f32 [S,D] partial output per core; copies round-robin ACT/DVE/Pool.
  - Emission interleaves projection strips with attention j-strips so
    the ACT exp stream starts ~25us in instead of after all projections.
"""

import sys

sys.path.insert(0, "/opt/trn_rl_repo")

import ml_dtypes
import numpy as np

import concourse.bass as bass  # noqa: F401
import concourse.bacc as bacc
import concourse.tile as tile
import concourse.mybir as mybir
from concourse.bass_utils import run_bass_kernel_spmd

F32 = mybir.dt.float32
BF16 = mybir.dt.bfloat16
AF = mybir.ActivationFunctionType
BF = ml_dtypes.bfloat16

B, S, D = 4, 2048, 1024
H, DH = 16, 64
HG = 8  # heads per core
DG = HG * DH  # 512 out-dims per core

_PROGRAM = None
LAST_RESULTS = None  # for test.py introspection


def _build_program():
    nc = bacc.Bacc("TRN2", target_bir_lowering=False, debug=False)

    xq_t = nc.dram_tensor("xq_t", [D, S], BF16, kind="ExternalInput")
    xk_t = nc.dram_tensor("xk_t", [D, S], BF16, kind="ExternalInput")
    xv_t = nc.dram_tensor("xv_t", [D, S], BF16, kind="ExternalInput")
    wq_t = nc.dram_tensor("wq_t", [D, DG], BF16, kind="ExternalInput")
    wk_t = nc.dram_tensor("wk_t", [D, DG], BF16, kind="ExternalInput")
    wv_t = nc.dram_tensor("wv_t", [D, DG], BF16, kind="ExternalInput")
    wo_t = nc.dram_tensor("wo_t", [DG, D], BF16, kind="ExternalInput")
    bq_c = nc.dram_tensor("bq_c", [128, 4], F32, kind="ExternalInput")
    bk_c = nc.dram_tensor("bk_c", [128, 4], F32, kind="ExternalInput")
    bv_r = nc.dram_tensor("bv_r", [1, DG], BF16, kind="ExternalInput")
    ones_b = nc.dram_tensor("ones_b", [1, 128], BF16, kind="ExternalInput")
    mask640 = nc.dram_tensor("mask640", [128, 640], BF16, kind="ExternalInput")
    out = nc.dram_tensor("out", [S, D], F32, kind="ExternalOutput")

    with tile.TileContext(nc) as tc:
        with (
            nc.allow_low_precision(reason="bf16 attention pipeline"),
            tc.tile_pool(name="persist", bufs=1) as pers,
        ):
            # ---- persistent tiles ----
            qT = [pers.tile([128, S], BF16, name=f"qT{i}") for i in range(4)]
            kT = [pers.tile([128, S], BF16, name=f"kT{i}") for i in range(4)]
            # v tiles: [128 s, 8 heads x (64 v + 1 ones)]
            vt = [pers.tile([128, HG * 65], BF16, name=f"v{i}") for i in range(16)]
            aout = [pers.tile([128, S], BF16, name=f"ao{i}") for i in range(4)]
            mask_sb = pers.tile([128, 640], BF16, name="mask640")
            ones_bf = pers.tile([65, 128], BF16, name="ones_bf")
            bq_sb = pers.tile([128, 4], F32, name="bq")
            bk_sb = pers.tile([128, 4], F32, name="bk")
            bv_sb = pers.tile([1, DG], BF16, name="bv")

            nc.sync.dma_start(out=mask_sb[:], in_=mask640[:])
            nc.sync.dma_start(out=ones_bf[0:1, :], in_=ones_b[:])
            nc.sync.dma_start(out=ones_bf[64:65, :], in_=ones_b[:])
            nc.sync.dma_start(out=bq_sb[:], in_=bq_c[:])
            nc.sync.dma_start(out=bk_sb[:], in_=bk_c[:])
            nc.sync.dma_start(out=bv_sb[:], in_=bv_r[:])

            # unified psum pool: tag "ps" [128,1024] bufs=3 (12KB/part),
            # tag "po" [65,512]/[64,512] bufs=2 (4KB/part) -> 16KB total
            pp = tc.alloc_tile_pool(name="pp", bufs=3, space="PSUM")

            with (
                tc.tile_pool(name="wbig", bufs=1) as wp,
                tc.tile_pool(name="xbig", bufs=4) as xp,
                tc.tile_pool(name="at", bufs=4) as ap_,
                tc.tile_pool(name="st", bufs=4) as stp,
                tc.tile_pool(name="rb", bufs=3) as rbp,
                tc.tile_pool(name="wo", bufs=4) as wop,
                tc.tile_pool(name="ob", bufs=3) as obp,
            ):
                # ---- weight tiles (each its own tag: all alive all of
                # phase 1 under the interleaved emission order). Loads go
                # through Pool DIRECT2D (~550GB/s, no ring serialization);
                # issue order on the Pool queue is the startup critical path:
                # wv -> xv0..3 -> wq/wk -> xq0/xk0 -> wo.
                wv_big = wp.tile([128, 8 * DG], BF16, tag="wv", name="wv_big")
                wq_big = wp.tile([128, 8 * DG], BF16, tag="wq", name="wq_big")
                wk_big = wp.tile([128, 8 * DG], BF16, tag="wk", name="wk_big")
                wo_sb = [
                    wop.tile([128, D], BF16, tag="wo", name=f"wo{c}")
                    for c in range(4)
                ]

                def load_w(w_sb, w_dr):
                    nc.gpsimd.dma_start(
                        out=w_sb[:].rearrange("p (k d) -> p k d", k=8),
                        in_=w_dr[:].rearrange("(k p) d -> p k d", p=128),
                    )

                load_w(wv_big, wv_t)

                xv_tiles = {}
                xqk_tiles = {}

                def load_xv(s):
                    xv_big = xp.tile([128, 8 * 128], BF16, tag="xvb", bufs=8, name="xv_big")
                    nc.gpsimd.dma_start(
                        out=xv_big[:].rearrange("p (k s2) -> p k s2", k=8),
                        in_=xv_t[:, s * 128 : (s + 1) * 128].rearrange(
                            "(k p) s2 -> p k s2", p=128
                        ),
                    )
                    xv_tiles[s] = xv_big

                def load_xqk(n):
                    xq_big = xp.tile([128, 8 * 512], BF16, tag="xb", bufs=4, name="xq_big")
                    xk_big = xp.tile([128, 8 * 512], BF16, tag="xb", bufs=4, name="xk_big")
                    nc.gpsimd.dma_start(
                        out=xq_big[:].rearrange("p (k s) -> p k s", k=8),
                        in_=xq_t[:, n * 512 : (n + 1) * 512].rearrange(
                            "(k p) s -> p k s", p=128
                        ),
                    )
                    nc.gpsimd.dma_start(
                        out=xk_big[:].rearrange("p (k s) -> p k s", k=8),
                        in_=xk_t[:, n * 512 : (n + 1) * 512].rearrange(
                            "(k p) s -> p k s", p=128
                        ),
                    )
                    xqk_tiles[n] = (xq_big, xk_big)

                def emit_v(s):
                    xv_big = xv_tiles.pop(s)
                    ps = pp.tile([128, 1024], F32, tag="ps", bufs=3, name="psv")
                    for k8 in range(8):
                        nc.tensor.matmul(
                            ps[:, 0:512],
                            xv_big[:, k8 * 128 : (k8 + 1) * 128],
                            wv_big[:, k8 * DG : (k8 + 1) * DG],
                            start=(k8 == 0),
                            stop=False,
                        )
                    nc.tensor.matmul(
                        ps[:, 0:512], ones_bf[0:1, :], bv_sb[:], start=False, stop=True
                    )
                    v3 = vt[s].rearrange("p (h x) -> p h x", x=65)
                    nc.vector.tensor_copy(
                        v3[:, :, 0:64],
                        ps[:, 0:512].rearrange("p (h d) -> p h d", d=64),
                    )

                def emit_qk(n):
                    xq_big, xk_big = xqk_tiles.pop(n)
                    for m in range(4):
                        ps = pp.tile([128, 1024], F32, tag="ps", bufs=3, name="psqk")
                        for w_big, x_big, half in (
                            (wq_big, xq_big, 0),
                            (wk_big, xk_big, 1),
                        ):
                            for k8 in range(8):
                                nc.tensor.matmul(
                                    ps[:, half * 512 : half * 512 + 512],
                                    w_big[
                                        :,
                                        k8 * DG + m * 128 : k8 * DG + (m + 1) * 128,
                                    ],
                                    x_big[:, k8 * 512 : (k8 + 1) * 512],
                                    start=(k8 == 0),
                                    stop=(k8 == 7),
                                )
                        # bias + cast + copy split across ACT and DVE so the
                        # psum slot frees fast (scores at the strip boundary
                        # otherwise wait on a backlogged DVE)
                        nc.scalar.activation(
                            qT[m][:, n * 512 : (n + 1) * 512],
                            ps[:, 0:512],
                            AF.Identity,
                            bias=bq_sb[:, m : m + 1],
                        )
                        nc.vector.tensor_scalar_add(
                            kT[m][:, n * 512 : (n + 1) * 512],
                            ps[:, 512:1024],
                            bk_sb[:, m : m + 1],
                        )

                def emit_norm(info):
                    # normalization for a PREVIOUS pair: by now its staging
                    # copies are long done, so the broadcast matmuls slot
                    # into the PE stream without stalling it
                    np_, nj, stA, stB = info
                    jc = slice(nj * 512, (nj + 1) * 512)
                    pbA = pp.tile([64, 512], F32, tag="ps", bufs=3, name="pbA")
                    nc.tensor.matmul(
                        pbA[:], ones_bf[64:65, 0:64], stA[64:65, :],
                        start=True, stop=True,
                    )
                    rbA = rbp.tile([64, 512], F32, tag="rb", name="rbA")
                    nc.vector.reciprocal_approx_fast(rbA[:], pbA[:])
                    nc.vector.tensor_mul(
                        aout[np_][0:64, jc], stA[0:64, :], rbA[:]
                    )
                    pbB = pp.tile([64, 512], F32, tag="ps", bufs=3, name="pbB")
                    nc.tensor.matmul(
                        pbB[:], ones_bf[64:65, 0:64], stB[64:65, :],
                        start=True, stop=True,
                    )
                    rbB = rbp.tile([64, 512], F32, tag="rb", name="rbB")
                    nc.vector.reciprocal_approx_fast(rbB[:], pbB[:])
                    nc.gpsimd.tensor_mul(stB[0:64, :], stB[0:64, :], rbB[:])
                    nc.gpsimd.dma_start(
                        out=aout[np_][64:128, jc], in_=stB[0:64, :]
                    )

                def emit_attn(p, j, norm_info):
                    hA, hB = 2 * p, 2 * p + 1
                    nsk = 4 * j + 4
                    ps_oA = pp.tile([65, 512], F32, tag="po", bufs=2, name="ps_oA")
                    ps_oB = pp.tile([65, 512], F32, tag="po", bufs=2, name="ps_oB")
                    pending = None
                    for i in range(nsk):
                        koff = i - 4 * j
                        c0 = 128 * koff if koff >= 0 else 0
                        ps = pp.tile([128, 1024], F32, tag="ps", bufs=3, name="ps_s")
                        nc.tensor.matmul(
                            ps[:, c0:512],
                            kT[p][0:64, i * 128 : (i + 1) * 128],
                            qT[p][0:64, j * 512 + c0 : (j + 1) * 512],
                            start=True,
                            stop=True,
                            tile_position=(0, 0),
                        )
                        nc.tensor.matmul(
                            ps[:, 512 + c0 : 1024],
                            kT[p][64:128, i * 128 : (i + 1) * 128],
                            qT[p][64:128, j * 512 + c0 : (j + 1) * 512],
                            start=True,
                            stop=True,
                            tile_position=(64, 0),
                        )
                        # retire previous iteration's PV while this exp runs
                        if pending is not None:
                            pi, pc0, pat = pending
                            nc.tensor.matmul(
                                ps_oA[:, pc0:512],
                                vt[pi][:, hA * 65 : hA * 65 + 65],
                                pat[:, pc0:512],
                                start=(pi == 0),
                                stop=False,
                            )
                            nc.tensor.matmul(
                                ps_oB[:, pc0:512],
                                vt[pi][:, hB * 65 : hB * 65 + 65],
                                pat[:, 512 + pc0 : 1024],
                                start=(pi == 0),
                                stop=False,
                            )
                        at = ap_.tile([128, 1024], BF16, tag="at", name="at")
                        nc.scalar.activation(
                            at[:, c0:1024], ps[:, c0:1024], AF.Exp, scale=0.125
                        )
                        if koff >= 0:
                            # zero below-diagonal in both heads' diag chunk
                            # with one [128,640] tri|ones|tri multiply
                            nc.vector.tensor_mul(
                                at[:, c0 : c0 + 640],
                                at[:, c0 : c0 + 640],
                                mask_sb[:],
                            )
                        pending = (i, c0, at)
                        if i == 2 and norm_info is not None:
                            emit_norm(norm_info)
                            norm_info = None
                    pi, pc0, pat = pending
                    nc.tensor.matmul(
                        ps_oA[:, pc0:512],
                        vt[pi][:, hA * 65 : hA * 65 + 65],
                        pat[:, pc0:512],
                        start=(pi == 0),
                        stop=True,
                    )
                    nc.tensor.matmul(
                        ps_oB[:, pc0:512],
                        vt[pi][:, hB * 65 : hB * 65 + 65],
                        pat[:, 512 + pc0 : 1024],
                        start=(pi == 0),
                        stop=True,
                    )
                    # raw bf16 staging (frees PSUM); normalization is
                    # deferred into the next pair's scores loop
                    stA = stp.tile([65, 512], BF16, tag="st", name="stA")
                    nc.vector.tensor_copy(stA[:], ps_oA[:])
                    stB = stp.tile([65, 512], BF16, tag="st", name="stB")
                    nc.vector.tensor_copy(stB[:], ps_oB[:])
                    return (p, j, stA, stB)

                # ---- interleaved emission: projections feed attention
                # j-strips as soon as their inputs exist; next strip's x
                # loads are issued before attention so the Pool drains them
                # while the PE chews on the current strip ----
                norm_info = None
                for j in range(4):
                    if j == 0:
                        for s in range(4):
                            load_xv(s)
                        load_w(wq_big, wq_t)
                        load_w(wk_big, wk_t)
                        load_xqk(0)
                    for s in range(4 * j, 4 * j + 4):
                        emit_v(s)
                    emit_qk(j)
                    if j < 3:
                        for s in range(4 * j + 4, 4 * j + 8):
                            load_xv(s)
                        load_xqk(j + 1)
                    if j == 0:
                        # vt ones columns (softmax denominator row): needed
                        # only by the first PV ~30us in; emitted after the
                        # startup-critical loads on the Pool queue
                        for s in range(16):
                            nc.gpsimd.memset(
                                vt[s].rearrange("p (h x) -> p h x", x=65)[
                                    :, :, 64:65
                                ],
                                1.0,
                            )
                        for c in range(4):
                            nc.gpsimd.dma_start(
                                out=wo_sb[c][:],
                                in_=wo_t[c * 128 : (c + 1) * 128, :],
                            )
                    for p in range(4):
                        norm_info = emit_attn(p, j, norm_info)

                if norm_info is not None:
                    emit_norm(norm_info)

                # ---- deferred Wo: accumulate all 4 pairs (K=512) ----
                ob_eng = [nc.scalar, nc.vector]
                for s in range(16):
                    psw = pp.tile([128, 1024], F32, tag="ps", bufs=3, name="psw")
                    for n2 in range(2):
                        for p in range(4):
                            nc.tensor.matmul(
                                psw[:, n2 * 512 : (n2 + 1) * 512],
                                aout[p][:, s * 128 : (s + 1) * 128],
                                wo_sb[p][:, n2 * 512 : (n2 + 1) * 512],
                                start=(p == 0),
                                stop=(p == 3),
                            )
                    ob = obp.tile([128, 1024], F32, tag="ob", name="ob")
                    eng = ob_eng[s % 2]
                    if eng is nc.scalar:
                        eng.copy(out=ob[:], in_=psw[:])
                    else:
                        eng.tensor_copy(ob[:], psw[:])
                    # split across rings; finer split for the last tiles so
                    # the end-of-kernel flush is short
                    nsplit = 4 if s >= 14 else 2
                    w = 1024 // nsplit
                    for q in range(nsplit):
                        nc.sync.dma_start(
                            out=out[s * 128 : (s + 1) * 128, q * w : (q + 1) * w],
                            in_=ob[:, q * w : (q + 1) * w],
                        )

            pp.release()

    nc.compile()
    return nc


def _make_in_maps(query, key, value, wq, bq, wk, bk, wv, bv, wo):
    f32 = np.float32
    ones_b = np.ones((1, 128), BF)
    # causal frame for diag chunks in scores_T layout: [tri | ones | tri]
    tri = np.triu(np.ones((128, 128), np.float32))
    mask640 = np.concatenate(
        [tri, np.ones((128, 384), np.float32), tri], axis=1
    ).astype(BF)

    wqT = np.asarray(wq, f32).T.astype(BF)  # [D, D] (d, dq)
    wkT = np.asarray(wk, f32).T.astype(BF)
    wvT = np.asarray(wv, f32).T.astype(BF)
    woT = np.asarray(wo, f32).T.astype(BF)  # [dv, D]

    in_maps = []
    for c in range(8):
        b, g = c // 2, c % 2
        sl = slice(g * DG, (g + 1) * DG)
        in_maps.append(
            {
                "xq_t": np.ascontiguousarray(np.asarray(query[b], f32).T.astype(BF)),
                "xk_t": np.ascontiguousarray(np.asarray(key[b], f32).T.astype(BF)),
                "xv_t": np.ascontiguousarray(np.asarray(value[b], f32).T.astype(BF)),
                "wq_t": np.ascontiguousarray(wqT[:, sl]),
                "wk_t": np.ascontiguousarray(wkT[:, sl]),
                "wv_t": np.ascontiguousarray(wvT[:, sl]),
                "wo_t": np.ascontiguousarray(woT[sl, :]),
                "bq_c": np.ascontiguousarray(
                    np.asarray(bq, f32)[sl].reshape(4, 128).T
                ),
                "bk_c": np.ascontiguousarray(
                    np.asarray(bk, f32)[sl].reshape(4, 128).T
                ),
                "bv_r": np.asarray(bv, f32)[sl].reshape(1, DG).astype(BF),
                "ones_b": ones_b,
                "mask640": mask640,
            }
        )
    return in_maps


def kernel(query, key, value, mask, wq, bq, wk, bk, wv, bv, wo, bo):
    global _PROGRAM, LAST_RESULTS
    if _PROGRAM is None:
        _PROGRAM = _build_program()
    nc = _PROGRAM
    in_maps = _make_in_maps(query, key, value, wq, bq, wk, bk, wv, bv, wo)

    res = run_bass_kernel_spmd(nc, in_maps, core_ids=list(range(8)))
    LAST_RESULTS = res

    f32 = np.float32
    outp = np.empty((B, S, D), f32)
    for b in range(B):
        outp[b] = res.results[2 * b]["out"] + res.results[2 * b + 1]["out"]
    outp += np.asarray(bo, f32)[None, None, :]
    return outp


# revision 55
# speedup vs baseline: 1.0164x; 1.0164x over previous
"""Multi-head attention (B=4, S=2048, D=1024, H=16, Dh=64) on 8 NeuronCores.

Sharding: core c handles batch b=c//2 and head-group g=c%2 (8 heads).
wq/wk/wv column-parallel, wo row-parallel; host sums the two partial
wo-products per batch and adds bo.

v2 design (per-core):
  - Projections (bf16 matmuls, f32 PSUM): V with ones-row bias matmul;
    Q/K packed q|k into one [128,1024] PSUM tile, bias+copy on Pool.
  - Causal flash attention per (head-pair, 512-wide q strip): scores for
    both heads land in one [128,1024] PSUM tile (A cols 0-511, B cols
    512-1023), ONE exp per k-block on ACT covering [c0:1024]; causal
    diag masked post-exp with a single [128,640] bf16 DVE multiply
    (tri|ones|tri frame). PV accumulates [v|1]^T @ at into [65,512]
    PSUM per head (row 64 = softmax denominator).
  - Normalization off the critical path: raw [65,512] PSUM -> bf16
    staging (frees PSUM), PE broadcast of the den row, DVE
    reciprocal_approx_fast, fused mul+cast into aout. Head B's half is
    moved to partitions 64-127 via a Pool DIRECT2D copy.
  - Wo deferred: accumulate all 4 head-pairs (K=512) in PSUM, single
    f32 [S,D] partial output per core; copies round-robin ACT/DVE/Pool.
  - Emission interleaves projection strips with attention j-strips so
    the ACT exp stream starts ~25us in instead of after all projections.
"""

import sys

sys.path.insert(0, "/opt/trn_rl_repo")

import ml_dtypes
import numpy as np

import concourse.bass as bass  # noqa: F401
import concourse.bacc as bacc
import concourse.tile as tile
import concourse.mybir as mybir
from concourse.bass_utils import run_bass_kernel_spmd

F32 = mybir.dt.float32
BF16 = mybir.dt.bfloat16
AF = mybir.ActivationFunctionType
BF = ml_dtypes.bfloat16

B, S, D = 4, 2048, 1024
H, DH = 16, 64
HG = 8  # heads per core
DG = HG * DH  # 512 out-dims per core

_PROGRAM = None
LAST_RESULTS = None  # for test.py introspection


def _build_program():
    nc = bacc.Bacc("TRN2", target_bir_lowering=False, debug=False)

    xq_t = nc.dram_tensor("xq_t", [D, S], BF16, kind="ExternalInput")
    xk_t = nc.dram_tensor("xk_t", [D, S], BF16, kind="ExternalInput")
    xv_t = nc.dram_tensor("xv_t", [D, S], BF16, kind="ExternalInput")
    wq_t = nc.dram_tensor("wq_t", [D, DG], BF16, kind="ExternalInput")
    wk_t = nc.dram_tensor("wk_t", [D, DG], BF16, kind="ExternalInput")
    wv_t = nc.dram_tensor("wv_t", [D, DG], BF16, kind="ExternalInput")
    wo_t = nc.dram_tensor("wo_t", [DG, D], BF16, kind="ExternalInput")
    bq_c = nc.dram_tensor("bq_c", [128, 4], F32, kind="ExternalInput")
    bk_c = nc.dram_tensor("bk_c", [128, 4], F32, kind="ExternalInput")
    bv_r = nc.dram_tensor("bv_r", [1, DG], BF16, kind="ExternalInput")
    ones_b = nc.dram_tensor("ones_b", [1, 128], BF16, kind="ExternalInput")
    mask640 = nc.dram_tensor("mask640", [128, 640], BF16, kind="ExternalInput")
    out = nc.dram_tensor("out", [S, D], F32, kind="ExternalOutput")

    with tile.TileContext(nc) as tc:
        with (
            nc.allow_low_precision(reason="bf16 attention pipeline"),
            tc.tile_pool(name="persist", bufs=1) as pers,
        ):
            # ---- persistent tiles ----
            qT = [pers.tile([128, S], BF16, name=f"qT{i}") for i in range(4)]
            kT = [pers.tile([128, S], BF16, name=f"kT{i}") for i in range(4)]
            # v tiles: [128 s, 8 heads x (64 v + 1 ones)]
            vt = [pers.tile([128, HG * 65], BF16, name=f"v{i}") for i in range(16)]
            aout = [pers.tile([128, S], BF16, name=f"ao{i}") for i in range(4)]
            mask_sb = pers.tile([128, 640], BF16, name="mask640")
            ones_bf = pers.tile([65, 128], BF16, name="ones_bf")
            bq_sb = pers.tile([128, 4], F32, name="bq")
            bk_sb = pers.tile([128, 4], F32, name="bk")
            bv_sb = pers.tile([1, DG], BF16, name="bv")

            nc.sync.dma_start(out=mask_sb[:], in_=mask640[:])
            nc.sync.dma_start(out=ones_bf[0:1, :], in_=ones_b[:])
            nc.sync.dma_start(out=ones_bf[64:65, :], in_=ones_b[:])
            nc.sync.dma_start(out=bq_sb[:], in_=bq_c[:])
            nc.sync.dma_start(out=bk_sb[:], in_=bk_c[:])
            nc.sync.dma_start(out=bv_sb[:], in_=bv_r[:])

            # unified psum pool: tag "ps" [128,1024] bufs=3 (12KB/part),
            # tag "po" [65,512]/[64,512] bufs=2 (4KB/part) -> 16KB total
            pp = tc.alloc_tile_pool(name="pp", bufs=3, space="PSUM")

            with (
                tc.tile_pool(name="wbig", bufs=1) as wp,
                tc.tile_pool(name="xbig", bufs=4) as xp,
                tc.tile_pool(name="at", bufs=4) as ap_,
                tc.tile_pool(name="st", bufs=4) as stp,
                tc.tile_pool(name="rb", bufs=3) as rbp,
                tc.tile_pool(name="wo", bufs=4) as wop,
                tc.tile_pool(name="ob", bufs=6) as obp,
            ):
                # ---- weight tiles (each its own tag: all alive all of
                # phase 1 under the interleaved emission order). Loads go
                # through Pool DIRECT2D (~550GB/s, no ring serialization);
                # issue order on the Pool queue is the startup critical path:
                # wv -> xv0..3 -> wq/wk -> xq0/xk0 -> wo.
                wv_big = wp.tile([128, 8 * DG], BF16, tag="wv", name="wv_big")
                wq_big = wp.tile([128, 8 * DG], BF16, tag="wq", name="wq_big")
                wk_big = wp.tile([128, 8 * DG], BF16, tag="wk", name="wk_big")
                wo_sb = [
                    wop.tile([128, D], BF16, tag="wo", name=f"wo{c}")
                    for c in range(4)
                ]

                def load_w(w_sb, w_dr):
                    nc.gpsimd.dma_start(
                        out=w_sb[:].rearrange("p (k d) -> p k d", k=8),
                        in_=w_dr[:].rearrange("(k p) d -> p k d", p=128),
                    )

                load_w(wv_big, wv_t)

                xv_tiles = {}
                xqk_tiles = {}

                def load_xv(s):
                    xv_big = xp.tile([128, 8 * 128], BF16, tag="xvb", bufs=8, name="xv_big")
                    nc.gpsimd.dma_start(
                        out=xv_big[:].rearrange("p (k s2) -> p k s2", k=8),
                        in_=xv_t[:, s * 128 : (s + 1) * 128].rearrange(
                            "(k p) s2 -> p k s2", p=128
                        ),
                    )
                    xv_tiles[s] = xv_big

                def load_xqk(n):
                    xq_big = xp.tile([128, 8 * 512], BF16, tag="xb", bufs=4, name="xq_big")
                    xk_big = xp.tile([128, 8 * 512], BF16, tag="xb", bufs=4, name="xk_big")
                    nc.gpsimd.dma_start(
                        out=xq_big[:].rearrange("p (k s) -> p k s", k=8),
                        in_=xq_t[:, n * 512 : (n + 1) * 512].rearrange(
                            "(k p) s -> p k s", p=128
                        ),
                    )
                    nc.gpsimd.dma_start(
                        out=xk_big[:].rearrange("p (k s) -> p k s", k=8),
                        in_=xk_t[:, n * 512 : (n + 1) * 512].rearrange(
                            "(k p) s -> p k s", p=128
                        ),
                    )
                    xqk_tiles[n] = (xq_big, xk_big)

                def emit_v(s):
                    xv_big = xv_tiles.pop(s)
                    ps = pp.tile([128, 1024], F32, tag="ps", bufs=3, name="psv")
                    for k8 in range(8):
                        nc.tensor.matmul(
                            ps[:, 0:512],
                            xv_big[:, k8 * 128 : (k8 + 1) * 128],
                            wv_big[:, k8 * DG : (k8 + 1) * DG],
                            start=(k8 == 0),
                            stop=False,
                        )
                    nc.tensor.matmul(
                        ps[:, 0:512], ones_bf[0:1, :], bv_sb[:], start=False, stop=True
                    )
                    v3 = vt[s].rearrange("p (h x) -> p h x", x=65)
                    nc.vector.tensor_copy(
                        v3[:, :, 0:64],
                        ps[:, 0:512].rearrange("p (h d) -> p h d", d=64),
                    )

                def emit_qk(n):
                    xq_big, xk_big = xqk_tiles.pop(n)
                    for m in range(4):
                        ps = pp.tile([128, 1024], F32, tag="ps", bufs=3, name="psqk")
                        for w_big, x_big, half in (
                            (wq_big, xq_big, 0),
                            (wk_big, xk_big, 1),
                        ):
                            for k8 in range(8):
                                nc.tensor.matmul(
                                    ps[:, half * 512 : half * 512 + 512],
                                    w_big[
                                        :,
                                        k8 * DG + m * 128 : k8 * DG + (m + 1) * 128,
                                    ],
                                    x_big[:, k8 * 512 : (k8 + 1) * 512],
                                    start=(k8 == 0),
                                    stop=(k8 == 7),
                                )
                        # bias + cast + copy split across ACT and DVE so the
                        # psum slot frees fast (scores at the strip boundary
                        # otherwise wait on a backlogged DVE)
                        nc.scalar.activation(
                            qT[m][:, n * 512 : (n + 1) * 512],
                            ps[:, 0:512],
                            AF.Identity,
                            bias=bq_sb[:, m : m + 1],
                        )
                        nc.vector.tensor_scalar_add(
                            kT[m][:, n * 512 : (n + 1) * 512],
                            ps[:, 512:1024],
                            bk_sb[:, m : m + 1],
                        )

                def emit_attn(p, j):
                    hA, hB = 2 * p, 2 * p + 1
                    nsk = 4 * j + 4
                    ps_oA = pp.tile([65, 512], F32, tag="po", bufs=2, name="ps_oA")
                    ps_oB = pp.tile([65, 512], F32, tag="po", bufs=2, name="ps_oB")
                    pending = None
                    for i in range(nsk):
                        koff = i - 4 * j
                        c0 = 128 * koff if koff >= 0 else 0
                        ps = pp.tile([128, 1024], F32, tag="ps", bufs=3, name="ps_s")
                        nc.tensor.matmul(
                            ps[:, c0:512],
                            kT[p][0:64, i * 128 : (i + 1) * 128],
                            qT[p][0:64, j * 512 + c0 : (j + 1) * 512],
                            start=True,
                            stop=True,
                            tile_position=(0, 0),
                        )
                        nc.tensor.matmul(
                            ps[:, 512 + c0 : 1024],
                            kT[p][64:128, i * 128 : (i + 1) * 128],
                            qT[p][64:128, j * 512 + c0 : (j + 1) * 512],
                            start=True,
                            stop=True,
                            tile_position=(64, 0),
                        )
                        # retire previous iteration's PV while this exp runs
                        if pending is not None:
                            pi, pc0, pat = pending
                            nc.tensor.matmul(
                                ps_oA[:, pc0:512],
                                vt[pi][:, hA * 65 : hA * 65 + 65],
                                pat[:, pc0:512],
                                start=(pi == 0),
                                stop=False,
                            )
                            nc.tensor.matmul(
                                ps_oB[:, pc0:512],
                                vt[pi][:, hB * 65 : hB * 65 + 65],
                                pat[:, 512 + pc0 : 1024],
                                start=(pi == 0),
                                stop=False,
                            )
                        at = ap_.tile([128, 1024], BF16, tag="at", name="at")
                        nc.scalar.activation(
                            at[:, c0:1024], ps[:, c0:1024], AF.Exp, scale=0.125
                        )
                        if koff >= 0:
                            # zero below-diagonal in both heads' diag chunk
                            # with one [128,640] tri|ones|tri multiply
                            nc.vector.tensor_mul(
                                at[:, c0 : c0 + 640],
                                at[:, c0 : c0 + 640],
                                mask_sb[:],
                            )
                        pending = (i, c0, at)
                    pi, pc0, pat = pending
                    nc.tensor.matmul(
                        ps_oA[:, pc0:512],
                        vt[pi][:, hA * 65 : hA * 65 + 65],
                        pat[:, pc0:512],
                        start=(pi == 0),
                        stop=True,
                    )
                    nc.tensor.matmul(
                        ps_oB[:, pc0:512],
                        vt[pi][:, hB * 65 : hB * 65 + 65],
                        pat[:, 512 + pc0 : 1024],
                        start=(pi == 0),
                        stop=True,
                    )
                    # raw bf16 staging (frees PSUM), then normalize
                    stA = stp.tile([65, 512], BF16, tag="st", name="stA")
                    nc.vector.tensor_copy(stA[:], ps_oA[:])
                    stB = stp.tile([65, 512], BF16, tag="st", name="stB")
                    nc.vector.tensor_copy(stB[:], ps_oB[:])
                    jc = slice(j * 512, (j + 1) * 512)
                    pbA = pp.tile([64, 512], F32, tag="po", bufs=2, name="pbA")
                    nc.tensor.matmul(
                        pbA[:], ones_bf[64:65, 0:64], stA[64:65, :],
                        start=True, stop=True,
                    )
                    rbA = rbp.tile([64, 512], F32, tag="rb", name="rbA")
                    nc.vector.reciprocal_approx_fast(rbA[:], pbA[:])
                    nc.vector.tensor_mul(aout[p][0:64, jc], stA[0:64, :], rbA[:])
                    pbB = pp.tile([64, 512], F32, tag="po", bufs=2, name="pbB")
                    nc.tensor.matmul(
                        pbB[:], ones_bf[64:65, 0:64], stB[64:65, :],
                        start=True, stop=True,
                    )
                    rbB = rbp.tile([64, 512], F32, tag="rb", name="rbB")
                    nc.vector.reciprocal_approx_fast(rbB[:], pbB[:])
                    nc.gpsimd.tensor_mul(stB[0:64, :], stB[0:64, :], rbB[:])
                    nc.gpsimd.dma_start(out=aout[p][64:128, jc], in_=stB[0:64, :])

                # ---- interleaved emission: projections feed attention
                # j-strips as soon as their inputs exist; next strip's x
                # loads are issued before attention so the Pool drains them
                # while the PE chews on the current strip ----
                for j in range(4):
                    if j == 0:
                        for s in range(4):
                            load_xv(s)
                        load_w(wq_big, wq_t)
                        load_w(wk_big, wk_t)
                        load_xqk(0)
                    for s in range(4 * j, 4 * j + 4):
                        emit_v(s)
                    emit_qk(j)
                    if j < 3:
                        for s in range(4 * j + 4, 4 * j + 8):
                            load_xv(s)
                        load_xqk(j + 1)
                    if j == 0:
                        # vt ones columns (softmax denominator row): needed
                        # only by the first PV ~30us in; emitted after the
                        # startup-critical loads on the Pool queue
                        for s in range(16):
                            nc.gpsimd.memset(
                                vt[s].rearrange("p (h x) -> p h x", x=65)[
                                    :, :, 64:65
                                ],
                                1.0,
                            )
                        for c in range(4):
                            nc.gpsimd.dma_start(
                                out=wo_sb[c][:],
                                in_=wo_t[c * 128 : (c + 1) * 128, :],
                            )
                    for p in range(4):
                        emit_attn(p, j)

                # ---- deferred Wo: accumulate all 4 pairs (K=512) ----
                ob_eng = [nc.scalar, nc.vector]
                for s in range(16):
                    psw = pp.tile([128, 1024], F32, tag="ps", bufs=3, name="psw")
                    for n2 in range(2):
                        for p in range(4):
                            nc.tensor.matmul(
                                psw[:, n2 * 512 : (n2 + 1) * 512],
                                aout[p][:, s * 128 : (s + 1) * 128],
                                wo_sb[p][:, n2 * 512 : (n2 + 1) * 512],
                                start=(p == 0),
                                stop=(p == 3),
                            )
                    ob = obp.tile([128, 1024], F32, tag="ob", name="ob")
                    eng = ob_eng[s % 2]
                    if eng is nc.scalar:
                        eng.copy(out=ob[:], in_=psw[:])
                    else:
                        eng.tensor_copy(ob[:], psw[:])
                    # split across rings; finer split for the last tiles so
                    # the end-of-kernel flush is short
                    nsplit = 4 if s >= 14 else 2
                    w = 1024 // nsplit
                    for q in range(nsplit):
                        nc.sync.dma_start(
                            out=out[s * 128 : (s + 1) * 128, q * w : (q + 1) * w],
                            in_=ob[:, q * w : (q + 1) * w],
                        )

            pp.release()

    nc.compile()
    return nc


def _make_in_maps(query, key, value, wq, bq, wk, bk, wv, bv, wo):
    f32 = np.float32
    ones_b = np.ones((1, 128), BF)
    # causal frame for diag chunks in scores_T layout: [tri | ones | tri]
    tri = np.triu(np.ones((128, 128), np.float32))
    mask640 = np.concatenate(
        [tri, np.ones((128, 384), np.float32), tri], axis=1
    ).astype(BF)

    wqT = np.asarray(wq, f32).T.astype(BF)  # [D, D] (d, dq)
    wkT = np.asarray(wk, f32).T.astype(BF)
    wvT = np.asarray(wv, f32).T.astype(BF)
    woT = np.asarray(wo, f32).T.astype(BF)  # [dv, D]

    in_maps = []
    for c in range(8):
        b, g = c // 2, c % 2
        sl = slice(g * DG, (g + 1) * DG)
        in_maps.append(
            {
                "xq_t": np.ascontiguousarray(np.asarray(query[b], f32).T.astype(BF)),
                "xk_t": np.ascontiguousarray(np.asarray(key[b], f32).T.astype(BF)),
                "xv_t": np.ascontiguousarray(np.asarray(value[b], f32).T.astype(BF)),
                "wq_t": np.ascontiguousarray(wqT[:, sl]),
                "wk_t": np.ascontiguousarray(wkT[:, sl]),
                "wv_t": np.ascontiguousarray(wvT[:, sl]),
                "wo_t": np.ascontiguousarray(woT[sl, :]),
                "bq_c": np.ascontiguousarray(
                    np.asarray(bq, f32)[sl].reshape(4, 128).T
                ),
                "bk_c": np.ascontiguousarray(
                    np.asarray(bk, f32)[sl].reshape(4, 128).T
                ),
                "bv_r": np.asarray(bv, f32)[sl].reshape(1, DG).astype(BF),
                "ones_b": ones_b,
                "mask640": mask640,
            }
        )
    return in_maps


def kernel(query, key, value, mask, wq, bq, wk, bk, wv, bv, wo, bo):
    global _PROGRAM, LAST_RESULTS
    if _PROGRAM is None:
        _PROGRAM = _build_program()
    nc = _PROGRAM
    in_maps = _make_in_maps(query, key, value, wq, bq, wk, bk, wv, bv, wo)

    res = run_bass_kernel_spmd(nc, in_maps, core_ids=list(range(8)))
    LAST_RESULTS = res

    f32 = np.float32
    outp = np.empty((B, S, D), f32)
    for b in range(B):
        outp[b] = res.results[2 * b]["out"] + res.results[2 * b + 1]["out"]
    outp += np.asarray(bo, f32)[None, None, :]
    return outp
